# revision 1
# baseline (speedup 1.0000x reference)
"""Trainium2 Bass kernel for nn_Attention_36481452212797.

Contract: kernel(**inputs) takes FULL inputs
  x [8, 4096, 256] f32, Wq/Wk/Wv [1024, 256], Wp [256, 1024], bp [256]
and returns the FULL output [8, 4096, 256] f32.

Sharding: data-parallel over B — one batch sample per NeuronCore, no
collectives. Per-core pipeline (per sample):

  xT = x.T                       (PE transposes)
  qT/q, kT/k = projections       (f32r matmuls, bf16 storage)
  DTA per stream (3-stage EM soft-clustering):
    bases0 = l2norm_c(maxpool32(qT))
    stage: zT = basesN.T @ qT    (bf16 MM, N=512)
           z  = softmax_KC(zT.T) (PE transpose + DVE/ACT)
           ybT = z.T @ q         (bf16 MM)
           basesT = l2norm_free(ybT)
    (the reference's l2norm of z over N cancels into the bases l2norm up
     to O(1e-8) — skipped)
  att_h = softmax_e(qbT_h.T @ kbT_h * SCALE)     (f32r)
  o_h   = attT_h.T @ vT_h                        (f32r, fused with final)
  out   = relu(o.T @ WpT + bp)                   (f32r, bias via K=1 matmul)

float32r is the PE's fast fp32 path (1 cycle/row at N>=256, ~1e-3 rel err);
bf16 is used only inside the DTA streams where the EM averaging washes the
rounding noise out (numpy-validated: end-to-end maxabs/scale ~3e-4).
"""

import copy
import sys
from contextlib import ExitStack

import numpy as np

sys.path.insert(0, "/opt/trn_rl_repo")

import concourse.bass as bass
import concourse.mybir as mybir
import concourse.tile as tile
from concourse.bass_utils import run_bass_kernel_spmd
from concourse.masks import make_identity

B, N, C, H, KC, STAGES = 8, 4096, 256, 8, 128, 3
C4 = 4 * C          # 1024
HD = C4 // H        # 128
SCALE = (C // H) ** -0.5
NT = N // 128       # 32 token tiles
NCH = C4 // 128     # 8 channel chunks
CCH = C // 128      # 2 input-channel chunks
W = N // KC         # 32: maxpool window

F32 = mybir.dt.float32
F32R = mybir.dt.float32r
BF16 = mybir.dt.bfloat16
AX = mybir.AxisListType
ALU = mybir.AluOpType
ACT = mybir.ActivationFunctionType


def cap_waits(nc, nop_templates, max_waits=1):
    """The walrus build here rejects instructions carrying more than one
    sync-wait command. Move excess waits onto EVSEM no-op carriers inserted
    before the capped instruction on the same engine."""
    m = nc.m
    new_m = copy.replace(m, functions=[])
    n_carriers = 0
    for function in m.functions:
        new_f = copy.replace(function, blocks=[])
        new_f.set_allocations_from_list(function.allocations)
        for block in function.blocks:
            new_insts = []
            for inst in block.instructions:
                si = inst.sync_info
                if si is not None and si.on_wait and len(si.on_wait) > max_waits:
                    waits = list(si.on_wait)
                    for w in waits[: len(waits) - max_waits]:
                        nop = copy.replace(
                            nop_templates[inst.engine],
                            name=f"{inst.name}-wc{n_carriers}",
                        )
                        tsi = nop_templates[inst.engine].sync_info
                        nop.sync_info = mybir.SyncInfo(
                            on_wait=[w],
                            on_update=list(tsi.on_update) if tsi else [],
                        )
                        new_insts.append(nop)
                        n_carriers += 1
                    inst.sync_info = mybir.SyncInfo(
                        on_wait=waits[len(waits) - max_waits :],
                        on_update=list(si.on_update or []),
                    )
                new_insts.append(inst)
            new_block = copy.replace(block, instructions=new_insts)
            new_f.blocks.append(new_block)
        new_m.functions.append(new_f)
    nc.m = new_m
    return n_carriers


def build_module():
    nc = bass.Bass()
    _dummy = nc.alloc_semaphore("waitcap_dummy")
    nop_templates = {
        e.ins.engine: e.ins
        for e in (
            nc.tensor.sem_inc(_dummy, 0),
            nc.vector.sem_inc(_dummy, 0),
            nc.scalar.sem_inc(_dummy, 0),
            nc.gpsimd.sem_inc(_dummy, 0),
            nc.sync.sem_inc(_dummy, 0),
        )
    }

    x_d = nc.declare_dram_parameter("x", [N, C], F32, isOutput=False)
    w_d = {
        "q": nc.declare_dram_parameter("Wq", [C4, C], F32, isOutput=False),
        "k": nc.declare_dram_parameter("Wk", [C4, C], F32, isOutput=False),
        "v": nc.declare_dram_parameter("Wv", [C4, C], F32, isOutput=False),
    }
    wp_d = nc.declare_dram_parameter("Wp", [C, C4], F32, isOutput=False)
    bp_d = nc.declare_dram_parameter("bp", [1, C], F32, isOutput=False)
    out_d = nc.declare_dram_parameter("out", [N, C], F32, isOutput=True)
    xT_dram = nc.dram_tensor("xT_scratch", [128, CCH * N], F32)

    with tile.TileContext(nc) as tc, ExitStack() as ctx:
        consts = ctx.enter_context(tc.tile_pool(name="consts", bufs=1))
        # PSUM: 3 + 3 + 2 banks = 8
        ps_mm = ctx.enter_context(tc.tile_pool(name="ps_mm", bufs=3, space="PSUM"))
        ps_tr = ctx.enter_context(tc.tile_pool(name="ps_tr", bufs=3, space="PSUM"))
        ps_sm = ctx.enter_context(tc.tile_pool(name="ps_sm", bufs=2, space="PSUM"))
        work = ctx.enter_context(tc.tile_pool(name="work", bufs=2))

        ident = consts.tile([128, 128], F32)
        make_identity(nc, ident[:])
        identr = consts.tile([128, 128], F32R)
        nc.vector.tensor_copy(identr[:], ident[:])

        ones_f = consts.tile([1, 128], F32)
        nc.vector.memset(ones_f[:], 1.0)
        ones_r = consts.tile([1, 128], F32R)
        nc.vector.tensor_copy(ones_r[:], ones_f[:])
        bp_f = consts.tile([1, C], F32)
        nc.sync.dma_start(bp_f[:], bp_d[:])
        bp_r = consts.tile([1, C], F32R)
        nc.vector.tensor_copy(bp_r[:], bp_f[:])

        qbT = consts.tile([128, C4], F32R, tag="qbT")
        kbT = consts.tile([128, C4], F32R, tag="kbT")

        def psum_copy(dst_ap, src_ap, idx, act_heavy=False):
            """Copy PSUM->SBUF alternating DVE/ACT to balance engine load.
            act_heavy routes 2/3 to ACT (projection phases keep DVE busy
            with reduces)."""
            dve = (idx % 6 == 0) if act_heavy else (idx % 2 == 0)
            if dve:
                nc.vector.tensor_copy(dst_ap, src_ap)
            else:
                nc.scalar.copy(dst_ap, src_ap)

        _tr_idx = [0]

        def pe_transpose(src_ap, dst_ap):
            """dst = src.T for one [128,128] fp32 block via PE."""
            ps = ps_tr.tile([128, 128], F32, tag="tr")
            nc.tensor.transpose(ps[:], src_ap, ident[:])
            _tr_idx[0] += 1
            psum_copy(dst_ap, ps[:], _tr_idx[0])

        def softmax_free(src_psum, out_ap, p, f, scale=1.0):
            """out = softmax over free axis of src_psum [p, f]. The inputs
            here are bounded (|logit| <= ~12), so the max-subtraction is
            skipped — exp stays comfortably inside fp32 range."""
            ex = work.tile([p, f], F32, tag="sm_exp", bufs=4)
            ssum = work.tile([p, 1], F32, tag="sm_sum", bufs=4)
            nc.scalar.activation(
                out=ex[:], in_=src_psum, func=ACT.Exp,
                scale=float(scale), accum_out=ssum[:],
            )
            rec = work.tile([p, 1], F32, tag="sm_rec", bufs=4)
            nc.vector.reciprocal(rec[:], ssum[:])
            nc.vector.tensor_scalar_mul(out_ap, ex[:], rec[:])

        def l2norm_free(src_ap, dst_ap, p, f):
            """dst = src / (1e-6 + l2norm of src row) over the free axis.
            sum(x^2) = f*(var + mean^2) via bn_stats (no big scratch)."""
            nsub = (f + 511) // 512
            sub = f // nsub
            src3 = src_ap.rearrange("p (n s) -> p n s", s=sub)
            stats = work.tile([p, nsub, 6], F32, tag="l2_stats")
            for i in range(nsub):
                nc.vector.bn_stats(out=stats[:, i, :], in_=src3[:, i, :])
            mv = work.tile([p, 2], F32, tag="l2_mv")
            nc.vector.bn_aggr(out=mv[:], in_=stats[:])
            m2 = work.tile([p, 1], F32, tag="l2_m2")
            nc.vector.tensor_mul(m2[:], mv[:, 0:1], mv[:, 0:1])
            nc.vector.tensor_add(m2[:], m2[:], mv[:, 1:2])
            nrm = work.tile([p, 1], F32, tag="l2_nrm")
            nc.scalar.activation(
                out=nrm[:], in_=m2[:], func=ACT.Sqrt, scale=float(f)
            )
            nc.vector.tensor_scalar_add(nrm[:], nrm[:], 1e-6)
            rec = work.tile([p, 1], F32, tag="l2_rec")
            nc.vector.reciprocal(rec[:], nrm[:])
            nc.vector.tensor_scalar_mul(dst_ap, src_ap, rec[:])

        def load_xT(pool, first):
            """First call: load x, transpose into xT [128, CCH, N] f32r and
            spill to DRAM. Later calls: reload the spilled copy."""
            xT = pool.tile([128, CCH, N], F32R, tag="xT")
            xT_flat = xT[:].rearrange("p a b -> p (a b)").bitcast(F32)
            Q = CCH * N // 4
            if first:
                for t4 in range(NT // 4):
                    xtile = work.tile([128, 4, C], F32, tag="ld")
                    eng = nc.sync if t4 % 2 == 0 else nc.gpsimd
                    eng.dma_start(
                        xtile[:],
                        x_d[bass.ds(t4 * 512, 512), :].rearrange(
                            "(a p) c -> p a c", p=128
                        ),
                    )
                    for a in range(4):
                        t = t4 * 4 + a
                        for j in range(CCH):
                            pe_transpose(
                                xtile[:, a, bass.ts(j, 128)],
                                xT[:, j, bass.ts(t, 128)],
                            )
                for i in range(4):
                    eng = nc.sync if i % 2 == 0 else nc.gpsimd
                    eng.dma_start(
                        xT_dram[:, bass.ds(i * Q, Q)], xT_flat[:, bass.ds(i * Q, Q)]
                    )
            else:
                for i in range(4):
                    eng = nc.sync if i % 2 == 0 else nc.gpsimd
                    eng.dma_start(
                        xT_flat[:, bass.ds(i * Q, Q)], xT_dram[:, bass.ds(i * Q, Q)]
                    )
            return xT

        def load_wT(pool, wd, dt=F32R):
            """Load one q/k/v weight and transpose into [128, CCH, C4]."""
            wT = pool.tile([128, CCH, C4], dt, tag="wT")
            for i2 in range(2):
                wtile = work.tile([128, 4, C], F32, tag="ld")
                eng = nc.sync if i2 % 2 == 0 else nc.gpsimd
                eng.dma_start(
                    wtile[:],
                    wd[bass.ds(i2 * 512, 512), :].rearrange("(a p) c -> p a c", p=128),
                )
                for a in range(4):
                    i = i2 * 4 + a
                    for j in range(CCH):
                        pe_transpose(
                            wtile[:, a, bass.ts(j, 128)], wT[:, j, bass.ts(i, 128)]
                        )
            return wT

        def projection_T(wT, xT_ap, dst_big, maxpool_to=None, t8s=None):
            """dst[c4, n] = W @ x.T as psum tiles [128, 512]. When
            maxpool_to is given, also reduce each psum tile over 32-token
            windows into it (bases0 seed, fused to overlap with the MMs)."""
            for i in range(NCH):
                for t8 in t8s if t8s is not None else range(N // 512):
                    ps = ps_mm.tile([128, 512], F32, tag="mm")
                    for j in range(CCH):
                        nc.tensor.matmul(
                            ps[:],
                            wT[:, j, bass.ts(i, 128)],
                            xT_ap(j, t8),
                            start=(j == 0),
                            stop=(j == CCH - 1),
                        )
                    psum_copy(
                        dst_big[:, i, bass.ds(t8 * 512, 512)], ps[:],
                        i + t8, act_heavy=True,
                    )
                    if maxpool_to is not None and t8 == (N // 512) - 1:
                        nc.vector.tensor_reduce(
                            maxpool_to[:, i, :],
                            dst_big[:, i, :].rearrange("p (k w) -> p k w", w=W),
                            axis=AX.X,
                            op=ALU.max,
                        )

        def projection_nat(wT, xT, dst_big):
            """dst[n, c4] = x @ W.T ; lhsT = xT tiles, rhs = WT chunks."""
            for t in range(NT):
                for c8 in range(C4 // 512):
                    ps = ps_mm.tile([128, 512], F32, tag="mm")
                    for j in range(CCH):
                        nc.tensor.matmul(
                            ps[:],
                            xT[:, j, bass.ts(t, 128)],
                            wT[:, j, bass.ds(c8 * 512, 512)],
                            start=(j == 0),
                            stop=(j == CCH - 1),
                        )
                    psum_copy(dst_big[:, t, bass.ds(c8 * 512, 512)], ps[:], t + c8, act_heavy=True)

        def dta_branch(stage_pool, sT_big, s_big, mx_big, out_basesT):
            """EM clustering on one stream; writes normalized bases (basesT
            layout [KC, C4]) into out_basesT (f32r). mx_big holds the fused
            maxpool seed from projection_T."""
            basesT = stage_pool.tile([128, C4], F32, tag="basesT")
            basesN = stage_pool.tile([128, NCH, 128], BF16, tag="basesN")
            z_big = stage_pool.tile([128, NT, KC], BF16, tag="z")

            for i in range(NCH):
                pe_transpose(mx_big[:, i, :], basesT[:, bass.ts(i, 128)])
            l2norm_free(basesT[:], basesT[:], 128, C4)

            for s in range(STAGES):
                # basesN <- basesT.T (bf16) for the stage-A matmul
                for i in range(NCH):
                    pe_transpose(basesT[:, bass.ts(i, 128)], basesN[:, i, :])

                # stage A: zT[k, n] = sum_c basesN[c,k] * sT[c,n];
                # then per 128-token block: PE transpose + softmax over KC
                for t8 in range(N // 512):
                    ps = ps_mm.tile([128, 512], F32, tag="mm")
                    for i in range(NCH):
                        nc.tensor.matmul(
                            ps[:],
                            basesN[:, i, :],
                            sT_big[:, i, bass.ds(t8 * 512, 512)],
                            start=(i == 0),
                            stop=(i == NCH - 1),
                        )
                    zst = work.tile([128, 512], F32R, tag="zstage")
                    nc.vector.tensor_copy(zst[:], ps[:])
                    for tt in range(4):
                        psz = ps_tr.tile([128, 128], F32R, tag="tr")
                        nc.tensor.matmul(
                            psz[:], zst[:, bass.ts(tt, 128)], identr[:],
                            is_transpose=True, start=True, stop=True,
                        )
                        softmax_free(psz[:], z_big[:, t8 * 4 + tt, :], 128, KC)

                # stage B: ybT[k, c] = sum_n z[n,k] * s[n,c]
                for c2 in range(C4 // 512):
                    ps = ps_mm.tile([128, 512], F32, tag="mm")
                    for t in range(NT):
                        nc.tensor.matmul(
                            ps[:],
                            z_big[:, t, :],
                            s_big[:, t, bass.ds(c2 * 512, 512)],
                            start=(t == 0),
                            stop=(t == NT - 1),
                        )
                    nc.vector.tensor_copy(
                        basesT[:, bass.ds(c2 * 512, 512)], ps[:]
                    )
                if s < STAGES - 1:
                    l2norm_free(basesT[:], basesT[:], 128, C4)
            l2norm_free(basesT[:], out_basesT, 128, C4)

        # ---- q and k branches (sequential; they share the big buffers) ----
        with ExitStack() as br_ctx:
            streams = br_ctx.enter_context(tc.tile_pool(name="streams", bufs=1))
            sT_big = streams.tile([128, NCH, N], BF16, tag="sT")
            s_big = streams.tile([128, NT, C4], BF16, tag="s_nat")
            mx_big = streams.tile([128, NCH, KC], F32, tag="mx")

            # q branch: f32r projection, builds + spills xT
            with ExitStack() as proj_ctx:
                ppool = proj_ctx.enter_context(tc.tile_pool(name="proj_q", bufs=1))
                wT = load_wT(ppool, w_d["q"])
                xT = load_xT(ppool, first=True)
                projection_T(
                    wT,
                    lambda j, t8: xT[:, j, bass.ds(t8 * 512, 512)],
                    sT_big,
                    maxpool_to=mx_big,
                )
                projection_nat(wT, xT, s_big)
            # bf16 copy of xT for the k projection, via casting SWDGE DMA
            # (runs during q's DTA while the DMA engines are idle; k only
            # feeds the error-tolerant EM clustering, bf16 is enough)
            xbf_pool = br_ctx.enter_context(tc.tile_pool(name="xbf", bufs=1))
            xTbf = xbf_pool.tile([128, CCH, N], BF16, tag="xTbf")
            xTbf_flat = xTbf[:].rearrange("p a b -> p (a b)")
            Q4 = CCH * N // 4
            for i in range(4):
                nc.gpsimd.dma_start(
                    xTbf_flat[:, bass.ds(i * Q4, Q4)],
                    xT_dram[:, bass.ds(i * Q4, Q4)],
                )
            with ExitStack() as st_ctx:
                stage_pool = st_ctx.enter_context(
                    tc.tile_pool(name="stage_q", bufs=1)
                )
                dta_branch(stage_pool, sT_big, s_big, mx_big, qbT[:])

            # k branch: all-bf16 projection from the resident xTbf
            with ExitStack() as proj_ctx:
                ppool = proj_ctx.enter_context(tc.tile_pool(name="proj_k", bufs=1))
                wTk = load_wT(ppool, w_d["k"], dt=BF16)
                projection_T(
                    wTk,
                    lambda j, t8: xTbf[:, j, bass.ds(t8 * 512, 512)],
                    sT_big,
                    maxpool_to=mx_big,
                )
                projection_nat(wTk, xTbf, s_big)
            with ExitStack() as st_ctx:
                stage_pool = st_ctx.enter_context(
                    tc.tile_pool(name="stage_k", bufs=1)
                )
                dta_branch(stage_pool, sT_big, s_big, mx_big, kbT[:])

        # ---- v projection, attention, output projection ----
        with ExitStack() as v_ctx:
            vpool = v_ctx.enter_context(tc.tile_pool(name="vpool", bufs=1))
            vT = vpool.tile([128, NCH, N], F32R, tag="vT")
            with ExitStack() as proj_ctx:
                ppool = proj_ctx.enter_context(tc.tile_pool(name="proj_v", bufs=1))
                wT = load_wT(ppool, w_d["v"])
                NH = N // 2
                for half in range(2):
                    xTh = ppool.tile([128, CCH, NH], F32R, tag="xTh")
                    xTh_flat = xTh[:].rearrange("p a b -> p (a b)").bitcast(F32)
                    for j in range(CCH):
                        eng = nc.sync if j % 2 == 0 else nc.gpsimd
                        eng.dma_start(
                            xTh_flat[:, bass.ds(j * NH, NH)],
                            xT_dram[:, bass.ds(j * N + half * NH, NH)],
                        )
                    projection_T(
                        wT,
                        lambda j, t8: xTh[:, j, bass.ds(t8 * 512 - half * NH, 512)],
                        vT,
                        t8s=range(half * 4, (half + 1) * 4),
                    )

            # WpT [128, NCH, C] f32r
            wpT = vpool.tile([128, NCH, C], F32R, tag="wpT")
            for i in range(CCH):
                for jj in range(4):
                    wtile = work.tile([128, C], F32, tag="ld")
                    nc.sync.dma_start(
                        wtile[:], wp_d[bass.ts(i, 128), bass.ds(jj * 256, 256)]
                    )
                    for j2 in range(2):
                        j = jj * 2 + j2
                        pe_transpose(
                            wtile[:, bass.ts(j2, 128)],
                            wpT[:, j, bass.ts(i, 128)],
                        )

            # attention per head: att = softmax_e(qh . kh^T * SCALE), then
            # transpose (f32r) for the o-matmul
            attT = vpool.tile([128, H, 128], F32R, tag="attT")
            att_s = vpool.tile([128, H, 128], F32R, tag="att_s")
            for h in range(H):
                psa = ps_sm.tile([128, 128], F32, tag="sm")
                nc.tensor.matmul(
                    psa[:],
                    qbT[:, bass.ts(h, 128)],
                    kbT[:, bass.ts(h, 128)],
                    start=True,
                    stop=True,
                )
                softmax_free(psa[:], att_s[:, h, :], 128, 128, scale=SCALE)
                pst = ps_tr.tile([128, 128], F32R, tag="tr")
                nc.tensor.matmul(
                    pst[:], att_s[:, h, :], identr[:],
                    is_transpose=True, start=True, stop=True,
                )
                nc.vector.tensor_copy(attT[:, h, :], pst[:])

            # o = attT.T @ vT, fused per 512-token chunk with the output
            # projection (+ bias via K=1 matmul) and relu
            oc_pool = v_ctx.enter_context(tc.tile_pool(name="oc", bufs=1))
            for t8 in range(N // 512):
                oc = oc_pool.tile([128, H, 512], F32R, tag="oc")
                for h in range(H):
                    ps = ps_mm.tile([128, 512], F32, tag="mm")
                    nc.tensor.matmul(
                        ps[:],
                        attT[:, h, :],
                        vT[:, h, bass.ds(t8 * 512, 512)],
                        start=True,
                        stop=True,
                    )
                    psum_copy(oc[:, h, :], ps[:], h)
                obig = work.tile([128, 4, C], F32, tag="obig")
                for tt in range(4):
                    pso = ps_sm.tile([128, C], F32, tag="sm")
                    for h in range(H):
                        nc.tensor.matmul(
                            pso[:],
                            oc[:, h, bass.ts(tt, 128)],
                            wpT[:, h, :],
                            start=(h == 0),
                            stop=False,
                        )
                    nc.tensor.matmul(
                        pso[:], ones_r[:], bp_r[:], start=False, stop=True
                    )
                    nc.scalar.activation(
                        out=obig[:, tt, :], in_=pso[:], func=ACT.Relu
                    )
                eng = nc.sync if t8 % 2 == 0 else nc.gpsimd
                eng.dma_start(
                    out_d[bass.ds(t8 * 512, 512), :].rearrange(
                        "(a p) c -> p a c", p=128
                    ),
                    obig[:],
                )

    cap_waits(nc, nop_templates)
    return nc


_NC_CACHE = None


def _get_module():
    global _NC_CACHE
    if _NC_CACHE is None:
        _NC_CACHE = build_module()
    return _NC_CACHE


def _in_maps(inputs):
    x = np.ascontiguousarray(inputs["x"], dtype=np.float32)
    shared = {
        "Wq": np.ascontiguousarray(inputs["Wq"], dtype=np.float32),
        "Wk": np.ascontiguousarray(inputs["Wk"], dtype=np.float32),
        "Wv": np.ascontiguousarray(inputs["Wv"], dtype=np.float32),
        "Wp": np.ascontiguousarray(inputs["Wp"], dtype=np.float32),
        "bp": np.ascontiguousarray(inputs["bp"], dtype=np.float32).reshape(1, C),
    }
    return [{"x": x[b], **shared} for b in range(B)]


def kernel(**inputs) -> np.ndarray:
    nc = _get_module()
    res = run_bass_kernel_spmd(nc, _in_maps(inputs), core_ids=list(range(B)))
    return np.stack([res.results[b]["out"] for b in range(B)], axis=0)


def run_traced(**inputs):
    """kernel() with NTFF tracing; returns (output, BassKernelResults)."""
    nc = _get_module()
    res = run_bass_kernel_spmd(
        nc, _in_maps(inputs), core_ids=list(range(B)), trace=True
    )
    out = np.stack([res.results[b]["out"] for b in range(B)], axis=0)
    return out, res



# revision 25
# speedup vs baseline: 1.9964x; 1.9964x over previous
"""Trainium2 Bass kernel for nn_Attention_36481452212797.

Contract: kernel(**inputs) takes FULL inputs
  x [8, 4096, 256] f32, Wq/Wk/Wv [1024, 256], Wp [256, 1024], bp [256]
and returns the FULL output [8, 4096, 256] f32.

Sharding: data-parallel over B — one batch sample per NeuronCore, no
collectives.

Algorithm (factored): q/k are rank-C products (q = x @ Wq.T), so every DTA
stage matmul is factored through the C=256 dim:
    z      = softmax_KC(x @ G)         with G = Wq.T @ bases      [C, KC]
    basesT = l2norm(S @ Wq.T)          with S = z.T @ x           [KC, C]
which cuts stage FLOPs ~3.2x vs materializing [N, 4C] streams. Only the
seed (bases0 = l2norm(maxpool(W @ x.T))) needs the full [4C, N] product.
The whole v-projection + cluster attention + output projection collapses
into a single [C, C] matrix
    P = sum_h Wp_h @ att_h @ Wv_h,   out = relu(x @ P.T + bp)
replacing ~5.4 GFLOP with ~0.74 GFLOP per sample.

The per-stage l2norm of bases is deferred: bases stay unnormalized and the
1/(1e-6+norm) factor rides as a per-partition scalar on the next stage's
G^T (partition axis = KC). The reference's l2norm of z over N cancels into
the bases l2norm (numpy-validated, O(1e-8)).

Precision: the big x-sided matmuls (seed, z, S^T, ybT, final) run in
fp8-e4m3 with DoubleRow perf mode (2 K-tiles per instruction); smaller
ones in bf16. fp8 operands carry power-of-2 scales (W^T x256, z x16,
S x64, P^T x32768) that are unwound exactly in ACT scale parameters.
End-to-end maxabs/scale vs the fp32 reference: ~2.3e-3 (numpy-validated).

Engine budget: the seed maxpool can ONLY run on DVE (Pool has no max
ucode and cannot touch PSUM; ACT has no max), so DVE gets the seed
reduces and softmax sums; ACT drains most other PSUM traffic (activation
`scale=` does the normalization muls for free); Pool does the SBUF-side
softmax muls via stride-0 broadcast.
"""

import copy
import sys
from contextlib import ExitStack

import numpy as np

sys.path.insert(0, "/opt/trn_rl_repo")

import concourse.bass as bass
import concourse.mybir as mybir
import concourse.tile as tile
from concourse.bass_utils import run_bass_kernel_spmd
from concourse.masks import make_identity

B, N, C, H, KC, STAGES = 8, 4096, 256, 8, 128, 3
C4 = 4 * C          # 1024
HD = C4 // H        # 128
SCALE = (C // H) ** -0.5
NT = N // 128       # 32 token tiles
W = N // KC         # 32: maxpool window

F32 = mybir.dt.float32
BF16 = mybir.dt.bfloat16
FP8 = mybir.dt.float8e4
AX = mybir.AxisListType
ALU = mybir.AluOpType
ACT = mybir.ActivationFunctionType
DR = mybir.MatmulPerfMode.DoubleRow
DS = bass.ds
TS = bass.ts

SW = 256.0     # wT8 = SW * W^T (fp8; seeds + z-logit matmuls only)
SZ = 1.0       # z stays bf16
SS = 1.0       # S^T stays bf16
SP = 1.0       # P^T stays bf16


def cap_waits(nc, nop_templates, max_waits=1):
    """The walrus build here rejects instructions carrying more than one
    sync-wait command. Move excess waits onto EVSEM no-op carriers inserted
    before the capped instruction on the same engine."""
    m = nc.m
    new_m = copy.replace(m, functions=[])
    n_carriers = 0
    for function in m.functions:
        new_f = copy.replace(function, blocks=[])
        new_f.set_allocations_from_list(function.allocations)
        for block in function.blocks:
            new_insts = []
            for inst in block.instructions:
                si = inst.sync_info
                if si is not None and si.on_wait and len(si.on_wait) > max_waits:
                    waits = list(si.on_wait)
                    for w in waits[: len(waits) - max_waits]:
                        nop = copy.replace(
                            nop_templates[inst.engine],
                            name=f"{inst.name}-wc{n_carriers}",
                        )
                        tsi = nop_templates[inst.engine].sync_info
                        nop.sync_info = mybir.SyncInfo(
                            on_wait=[w],
                            on_update=list(tsi.on_update) if tsi else [],
                        )
                        new_insts.append(nop)
                        n_carriers += 1
                    inst.sync_info = mybir.SyncInfo(
                        on_wait=waits[len(waits) - max_waits :],
                        on_update=list(si.on_update or []),
                    )
                new_insts.append(inst)
            new_block = copy.replace(block, instructions=new_insts)
            new_f.blocks.append(new_block)
        new_m.functions.append(new_f)
    nc.m = new_m
    return n_carriers


def build_module():
    nc = bass.Bass()
    _dummy = nc.alloc_semaphore("waitcap_dummy")
    nop_templates = {
        e.ins.engine: e.ins
        for e in (
            nc.tensor.sem_inc(_dummy, 0),
            nc.vector.sem_inc(_dummy, 0),
            nc.scalar.sem_inc(_dummy, 0),
            nc.gpsimd.sem_inc(_dummy, 0),
            nc.sync.sem_inc(_dummy, 0),
        )
    }

    x_d = nc.declare_dram_parameter("x", [N, C], F32, isOutput=False)
    w_d = {
        "q": nc.declare_dram_parameter("Wq", [C4, C], F32, isOutput=False),
        "k": nc.declare_dram_parameter("Wk", [C4, C], F32, isOutput=False),
        "v": nc.declare_dram_parameter("Wv", [C4, C], F32, isOutput=False),
    }
    wp_d = nc.declare_dram_parameter("Wp", [C, C4], F32, isOutput=False)
    bp_d = nc.declare_dram_parameter("bp", [1, C], F32, isOutput=False)
    out_d = nc.declare_dram_parameter("out", [N, C], F32, isOutput=True)
    DBG = True
    if DBG:
        dbg = {
            nm: nc.declare_dram_parameter(f"dbg_{nm}", [128, sz], F32, isOutput=True)
            for nm, sz in [
                ("mxq", 8 * KC), ("bnrq", C4), ("g8q", 2 * KC), ("sumq", NT),
                ("ztq", NT * KC), ("st8q", 2 * KC), ("qbT", C4), ("kbT", C4),
                ("pt8", 2 * C),
            ]
        }

    with tile.TileContext(nc) as tc, ExitStack() as ctx:
        consts = ctx.enter_context(tc.tile_pool(name="consts", bufs=1))
        main = ctx.enter_context(tc.tile_pool(name="main", bufs=1))
        wk = ctx.enter_context(tc.tile_pool(name="wk", bufs=2))

        ident_f = consts.tile([128, 128], F32)
        make_identity(nc, ident_f[:])
        ident_b = consts.tile([128, 128], BF16)
        nc.vector.tensor_copy(ident_b[:], ident_f[:])
        eps_ap = {}
        for nm, v in (("sw", SW * 1e-6), ("yb", 1e-6)):
            t = consts.tile([128, 1], F32, name=f"eps_{nm}", tag=f"eps_{nm}")
            nc.vector.memset(t[:], float(v))
            eps_ap[nm] = t

        ones_b = consts.tile([1, 128], BF16)
        nc.vector.memset(ones_b[:], 1.0)
        bp_f = consts.tile([1, C], F32)
        nc.sync.dma_start(bp_f[:], bp_d[:])
        bp_sb = consts.tile([1, C], BF16)
        with nc.allow_low_precision(reason="bias prescale"):
            nc.vector.tensor_scalar_mul(bp_sb[:], bp_f[:], float(SP))

        # ---- persistent tensors ----
        x8 = main.tile([128, NT, C], FP8, tag="x8")
        xb = main.tile([128, NT, C], BF16, tag="xb")
        xT8 = main.tile([128, 2, N], FP8, tag="xT8")
        xTb = main.tile([128, 2, N], BF16, tag="xTb")
        wnat = {s: main.tile([128, 8, C], BF16, tag=f"wnat_{s}", name=f"wnat_{s}") for s in "qk"}
        wT8 = {s: main.tile([128, 2, C4], FP8, tag=f"wT8_{s}", name=f"wT8_{s}") for s in "qk"}
        wTb = {s: main.tile([128, 2, C4], BF16, tag=f"wTb_{s}", name=f"wTb_{s}") for s in "qk"}
        wv = main.tile([128, 8, C], BF16, tag="wv")
        wpT = main.tile([128, 8, C], BF16, tag="wpT")
        outbT = {"q": main.tile([128, C4], BF16, tag="qbT", name="qbT"),
                 "k": main.tile([128, C4], BF16, tag="kbT", name="kbT")}
        mx = {s: main.tile([128, 8, KC], BF16, tag=f"mx_{s}", name=f"mx_{s}") for s in "qk"}
        zt = {s: main.tile([128, NT, KC], BF16, tag=f"z_{s}", name=f"z_{s}") for s in "qk"}
        ext = {s: main.tile([128, NT, KC], BF16, tag=f"ex_{s}", name=f"ex_{s}") for s in "qk"}
        bnr = {s: main.tile([128, C4], BF16, tag=f"bnr_{s}", name=f"bnr_{s}") for s in "qk"}
        basesN = {s: main.tile([128, 8, KC], BF16, tag=f"bN_{s}", name=f"bN_{s}") for s in "qk"}
        sqscr = {s: main.tile([128, C4], BF16, tag=f"sq_{s}", name=f"sq_{s}") for s in "qk"}
        G8 = {s: main.tile([128, 2, KC], FP8, tag=f"G_{s}", name=f"G_{s}") for s in "qk"}
        ST8 = {s: main.tile([128, 2, KC], BF16, tag=f"ST_{s}", name=f"ST_{s}") for s in "qk"}
        sums = {s: main.tile([128, NT], BF16, tag=f"sm_{s}", name=f"sm_{s}") for s in "qk"}
        recb = {s: main.tile([128, NT], BF16, tag=f"rb_{s}", name=f"rb_{s}") for s in "qk"}
        ssq = {s: main.tile([128, 1], F32, tag=f"ssq_{s}", name=f"ssq_{s}") for s in "qk"}
        rnrm = {s: main.tile([128, 1], F32, tag=f"rn_{s}", name=f"rn_{s}") for s in "qk"}
        rnrmG = {s: main.tile([128, 1], F32, tag=f"rg_{s}", name=f"rg_{s}") for s in "qk"}

        _rot = [0]

        def rot_copy(dst_ap, src_ap, engines="vs", scale=None):
            """Rotate PSUM->SBUF drains across DVE(v)/ACT(s)."""
            e = engines[_rot[0] % len(engines)]
            _rot[0] += 1
            if scale is not None:
                if e == "v":
                    nc.vector.tensor_scalar_mul(dst_ap, src_ap, float(scale))
                else:
                    nc.scalar.activation(
                        out=dst_ap, in_=src_ap, func=ACT.Copy, scale=float(scale)
                    )
            elif e == "v":
                nc.vector.tensor_copy(dst_ap, src_ap)
            else:
                nc.scalar.copy(dst_ap, src_ap)

        # ================= phase 0: loads + transposes =================
        with ExitStack() as p0:
            tr8 = p0.enter_context(tc.tile_pool(name="tr8", bufs=2, space="PSUM"))
            trb = p0.enter_context(tc.tile_pool(name="trb", bufs=2, space="PSUM"))
            trf = p0.enter_context(tc.tile_pool(name="trf", bufs=2, space="PSUM"))

            # loads (gpsimd = SWDGE casting DMA; sync = plain f32)
            def load_x(c):
                nc.gpsimd.dma_start(
                    xb[:, DS(c * 8, 8), :],
                    x_d[DS(c * 1024, 1024), :].rearrange("(a p) c -> p a c", p=128),
                )
                nc.gpsimd.dma_start(
                    x8[:, DS(c * 8, 8), :],
                    x_d[DS(c * 1024, 1024), :].rearrange("(a p) c -> p a c", p=128),
                )

            def load_w(s, dst):
                nc.gpsimd.dma_start(
                    dst, w_d[s][:].rearrange("(a p) c -> p a c", p=128)
                )

            load_x(0)
            load_x(1)
            load_w("q", wnat["q"][:])
            load_x(2)
            load_x(3)
            load_w("k", wnat["k"][:])
            load_w("v", wv[:])
            wp_stage = wk.tile([128, 2, C4], F32, tag="wp_stage", bufs=1)
            nc.sync.dma_start(
                wp_stage[:], wp_d[:].rearrange("(a p) c -> p a c", p=128)
            )

            # x transposes: xT8[:, j, t*128:(t+1)*128] = x8[:, t, j*128:..]^T
            for c in range(4):
                for j in range(2):
                    ps = tr8.tile([128, 1024], BF16, tag="tr8")
                    for a in range(8):
                        t = c * 8 + a
                        nc.tensor.transpose(
                            ps[:, TS(a, 128)], xb[:, t, TS(j, 128)], ident_b[:]
                        )
                    rot_copy(xT8[:, j, DS(c * 1024, 1024)], ps[:])
                    rot_copy(xTb[:, j, DS(c * 1024, 1024)], ps[:])

            # weight transposes -> fp8 with xSW scale
            for s in "qk":
                for jj in range(2):
                    ps = trb.tile([128, 1024], BF16, tag="trb")
                    for i in range(8):
                        nc.tensor.transpose(
                            ps[:, TS(i, 128)], wnat[s][:, i, TS(jj, 128)], ident_b[:]
                        )
                    rot_copy(wT8[s][:, jj, :], ps[:], scale=SW)
                    rot_copy(wTb[s][:, jj, :], ps[:])

            # Wp transposes: wpT[:, j, a*128:..] = wp_stage[:, a, j*128:..]^T
            for a in range(2):
                for jg in range(2):
                    ps = trf.tile([128, 4, 128], F32, tag="trf")
                    for u in range(4):
                        j = jg * 4 + u
                        nc.tensor.transpose(
                            ps[:, u, :], wp_stage[:, a, TS(j, 128)], ident_f[:]
                        )
                    rot_copy(wpT[:, DS(jg * 4, 4), TS(a, 128)], ps[:])

        # ========= phases 1+2: seeds + DTA stages (one PSUM layout) =====
        # The seed maxpool is a 76us DVE-serial job; to keep DVE dense the
        # k-stream's seed groups are interleaved into q's stage pipeline
        # (and the stages of the two streams are interleaved with each
        # other). One shared PSUM layout (8 banks):
        #   sdyb [128,1024]f32 x2 (seed groups + ybT)  = 4 banks
        #   trb2 [128,1024]bf16 x1, tg8/zp/stp [128,512] x1 each = 4 banks
        with ExitStack() as p12:
            sdyb = p12.enter_context(tc.tile_pool(name="sdyb", bufs=2, space="PSUM"))
            trb2 = p12.enter_context(tc.tile_pool(name="trb2", bufs=1, space="PSUM"))
            zp = p12.enter_context(tc.tile_pool(name="zp", bufs=2, space="PSUM"))
            stp = p12.enter_context(tc.tile_pool(name="stp", bufs=1, space="PSUM"))

            def seed_group(s, m, qtr):
                """One [128,1024] seed psum group + its DVE maxpool reduce."""
                ps = sdyb.tile([128, 1024], F32, tag="sdyb", name="ps_sd")
                for u in range(2):
                    nc.tensor.matmul(
                        ps[:, TS(u, 512)],
                        wT8[s][:, :, TS(m, 128)],
                        xT8[:, :, DS(qtr * 1024 + u * 512, 512)],
                        start=True,
                        stop=True,
                        perf_mode=DR,
                    )
                nc.vector.tensor_reduce(
                    mx[s][:, m, DS(qtr * 32, 32)],
                    ps[:].rearrange("p (k w) -> p k w", w=W),
                    axis=AX.X,
                    op=ALU.max,
                )

            def norm_chain(s, src_ap, sc, eps):
                """rnrmG[s] = (1/sc)/(1e-6 + sqrt(sum(src^2))/sc) — the drain
                scale yielding l2-normalized bases from an sc-scaled source
                (per-partition AP; partition = KC). 3 ACT ops, no engine
                hops: 1/(u + sc*eps) == (1/sc)/(eps + u/sc) for u=sc*nrm."""
                nc.scalar.activation(
                    out=sqscr[s][:], in_=src_ap, func=ACT.Square,
                    accum_out=ssq[s][:],
                )
                nc.scalar.activation(
                    out=rnrm[s][:], in_=ssq[s][:], func=ACT.Sqrt, bias=eps[:]
                )
                nc.vector.reciprocal(rnrmG[s][:], rnrm[s][:])

            def bases0(s):
                """mx (SW-scaled) -> bnr = l2-normalized basesT. The norm is
                computed from the raw transposed copy, then applied in
                place (per-partition scalar; partition = KC)."""
                ps = trb2.tile([128, 1024], BF16, tag="trb2")
                for i in range(8):
                    nc.tensor.transpose(
                        ps[:, TS(i, 128)], mx[s][:, i, :], ident_b[:]
                    )
                nc.scalar.copy(bnr[s][:], ps[:])
                norm_chain(s, bnr[s][:], SW, eps_ap["sw"])
                nc.vector.tensor_scalar_mul(bnr[s][:], bnr[s][:], rnrmG[s][:])

            def pool_sum_tree(s, p):
                """Softmax denominators for 8 token tiles on Pool (7-level
                binary adds) to offload DVE."""
                t1 = wk.tile([128, 8, 64], BF16, tag="t1", bufs=2, name="t1")
                t2 = wk.tile([128, 8, 32], BF16, tag="t2", bufs=2, name="t2")
                t3 = wk.tile([128, 8, 16], BF16, tag="t3", bufs=2, name="t3")
                t4 = wk.tile([128, 8, 8], BF16, tag="t4", bufs=2, name="t4")
                t5 = wk.tile([128, 8, 4], BF16, tag="t5", bufs=2, name="t5")
                t6 = wk.tile([128, 8, 2], BF16, tag="t6", bufs=2, name="t6")
                ex8 = ext[s][:, DS(p * 8, 8), :]
                ad = nc.gpsimd.tensor_add
                with nc.allow_low_precision(reason="softmax denom"):
                    ad(t1[:], ex8[:, :, 0:64], ex8[:, :, 64:128])
                    ad(t2[:], t1[:, :, 0:32], t1[:, :, 32:64])
                    ad(t3[:], t2[:, :, 0:16], t2[:, :, 16:32])
                    ad(t4[:], t3[:, :, 0:8], t3[:, :, 8:16])
                    ad(t5[:], t4[:, :, 0:4], t4[:, :, 4:8])
                    ad(t6[:], t5[:, :, 0:2], t5[:, :, 2:4])
                    ad(sums[s][:, DS(p * 8, 8)], t6[:, :, 0], t6[:, :, 1])

            def stage(s, i, ve, mode="spread"):
                # hide: everything off-DVE where possible except sums
                # spread: sums alternate DVE/Pool-tree, muls all on Pool
                hide_ = mode == "hide"
                """Generator: yields at pipeline checkpoints so the caller
                can interleave other work into the emission order. `ve` is
                the drain engine (nc.vector or nc.scalar) for this stage.

                bnr holds l2-NORMALIZED basesT (the norm was folded into
                its drain), so G8 = SW/4 * sum_ch W_ch.T @ bases_ch comes
                straight out of a matmul with a plain scaled drain.
                """
                last = i == STAGES - 1
                hide = hide_

                def drain(dst, src, scale=None):
                    if scale is None:
                        if ve is nc.vector:
                            nc.vector.tensor_copy(dst, src)
                        else:
                            nc.scalar.copy(dst, src)
                    elif ve is nc.vector:
                        nc.vector.tensor_scalar_mul(dst, src, scale)
                    else:
                        nc.scalar.activation(
                            out=dst, in_=src, func=ACT.Copy, scale=scale
                        )

                # -- basesN [c4, KC] = transpose(bnr) --
                ps = trb2.tile([128, 1024], BF16, tag="trb2")
                for ch in range(8):
                    nc.tensor.transpose(
                        ps[:, TS(ch, 128)], bnr[s][:, TS(ch, 128)], ident_b[:]
                    )
                drain(basesN[s][:].rearrange("p a k -> p (a k)"), ps[:])
                # -- G8 = SG * sum_ch W_ch.T @ basesN_ch  [C, KC] --
                gps = zp.tile([128, 512], F32, tag="zp")
                for j in range(2):
                    for ch in range(8):
                        nc.tensor.matmul(
                            gps[:, DS(j * 128, 128)],
                            wnat[s][:, ch, TS(j, 128)],
                            basesN[s][:, ch, :],
                            start=(ch == 0),
                            stop=(ch == 7),
                        )
                drain(
                    G8[s][:].rearrange("p a k -> p (a k)"), gps[:, 0:256],
                    scale=float(SW / 4.0),
                )
                yield
                # -- z8 = SZ * softmax_KC(x @ G); z psum carries SW --
                for g in range(8):
                    ps = zp.tile([128, 512], F32, tag="zp")
                    for tt in range(4):
                        t = g * 4 + tt
                        nc.tensor.matmul(
                            ps[:, TS(tt, 128)],
                            xT8[:, :, TS(t, 128)],
                            G8[s][:],
                            start=True,
                            stop=True,
                            perf_mode=DR,
                        )
                    nc.scalar.activation(
                        out=ext[s][:, DS(g * 4, 4), :], in_=ps[:], func=ACT.Exp,
                        scale=float(4.0 / SW),
                    )
                    if g % 2 == 1:
                        p = g // 2
                        if not hide and p % 2 == 1:
                            pool_sum_tree(s, p)
                        else:
                            with nc.allow_low_precision(reason="softmax denom"):
                                nc.vector.tensor_reduce(
                                    sums[s][:, DS(p * 8, 8)],
                                    ext[s][:, DS(p * 8, 8), :],
                                    axis=AX.X,
                                    op=ALU.add,
                                )
                        with nc.allow_low_precision(reason="softmax recip"):
                            nc.vector.reciprocal(
                                recb[s][:, DS(p * 8, 8)], sums[s][:, DS(p * 8, 8)]
                            )
                            nc.vector.tensor_scalar_mul(
                                recb[s][:, DS(p * 8, 8)],
                                recb[s][:, DS(p * 8, 8)],
                                float(SZ),
                            )
                        for hh in range(2):
                            o = p * 8 + hh * 4
                            rb = (
                                recb[s][:, DS(o, 4)]
                                .rearrange("p a -> p a ()")
                                .broadcast_to((128, 4, KC))
                            )
                            nc.gpsimd.tensor_mul(
                                zt[s][:, DS(o, 4), :],
                                ext[s][:, DS(o, 4), :],
                                rb,
                            )
                        yield
                # -- ST8 = SS * (sum_t x_t.T @ z_t); S psum carries SZ --
                sps = stp.tile([128, 512], F32, tag="stp")
                for t in range(NT):
                    for j in range(2):
                        nc.tensor.matmul(
                            sps[:, TS(j, 128)],
                            xb[:, t, TS(j, 128)],
                            zt[s][:, t, :],
                            start=(t == 0),
                            stop=(t == NT - 1),
                        )
                nc.scalar.copy(
                    ST8[s][:].rearrange("p a k -> p (a k)"), sps[:, 0:256]
                )
                yield

                # -- ybT = S @ W.T [KC, C4]; psum carries SS*SW --
                yps = sdyb.tile([128, 1024], F32, tag="sdyb", name="ps_yb")
                for hh in range(2):
                    for j in range(2):
                        nc.tensor.matmul(
                            yps[:, TS(hh, 512)],
                            ST8[s][:, j, :],
                            wTb[s][:, j, DS(hh * 512, 512)],
                            start=(j == 0),
                            stop=(j == 1),
                        )
                ysc = 1.0
                norm_chain(s, yps[:], ysc, eps_ap["yb"])
                dst = outbT[s] if last else bnr[s]
                if ve is nc.vector:
                    nc.vector.tensor_scalar_mul(dst[:], yps[:], rnrmG[s][:])
                else:
                    nc.scalar.activation(
                        out=dst[:], in_=yps[:], func=ACT.Copy, scale=rnrmG[s][:]
                    )
                yield

            def pump(gen, n=1):
                """Advance a stage generator n checkpoints; True if alive."""
                if gen is None:
                    return None
                for _ in range(n):
                    if next(gen, "done") == "done":
                        return None
                return gen

            # q seeds first, then q's stages overlap k's seeds; the two
            # streams' stages interleave checkpoint-by-checkpoint after.
            kseeds = [(m, qtr) for qtr in range(4) for m in range(8)]
            for qtr in range(4):
                for m in range(8):
                    seed_group("q", m, qtr)
            if DBG:
                nc.gpsimd.dma_start(dbg["mxq"][:], mx["q"][:].rearrange("p a k -> p (a k)"))
            bases0("q")
            if DBG:
                nc.gpsimd.dma_start(dbg["bnrq"][:], bnr["q"][:])
            gq = stage("q", 0, nc.scalar, "hide")
            for gi, (m, qtr) in enumerate(kseeds):
                seed_group("k", m, qtr)
                if gi % 2 == 1 and gq is not None:
                    gq = pump(gq)
            while gq is not None:
                gq = pump(gq)
            if DBG:
                nc.gpsimd.dma_start(dbg["g8q"][:], G8["q"][:].rearrange("p a k -> p (a k)"))
                nc.gpsimd.dma_start(dbg["sumq"][:], sums["q"][:])
                nc.gpsimd.dma_start(dbg["ztq"][:], zt["q"][:].rearrange("p a k -> p (a k)"))
                nc.gpsimd.dma_start(dbg["st8q"][:], ST8["q"][:].rearrange("p a k -> p (a k)"))
            bases0("k")
            pairs = [("k", 0, "q", 1), ("k", 1, "q", 2)]
            for sa, ia, sb, ib in pairs:
                ga = stage(sa, ia, nc.scalar, "spread")
                gb = stage(sb, ib, nc.scalar, "spread")
                ga = pump(ga, 3)
                while ga is not None or gb is not None:
                    ga = pump(ga)
                    gb = pump(gb)
            gk = stage("k", 2, nc.scalar, "spread")
            while gk is not None:
                gk = pump(gk)

        # ================= phase 3: attention + P + final ==============
        with ExitStack() as p3:
            pf = p3.enter_context(tc.tile_pool(name="pf", bufs=3, space="PSUM"))
            trr = p3.enter_context(tc.tile_pool(name="trr", bufs=2, space="PSUM"))

            ex_att = wk.tile([128, 8, KC], BF16, tag="ex_att", bufs=1)
            asum = wk.tile([128, 8], BF16, tag="asum", bufs=1)
            arec = wk.tile([128, 8], F32, tag="arec", bufs=1)
            att_s = wk.tile([128, 8, KC], BF16, tag="att_s", bufs=1)
            attT = wk.tile([128, 8, KC], BF16, tag="attT", bufs=1)
            Bt = wk.tile([128, 8, C], BF16, tag="Bt", bufs=1)
            P_s = wk.tile([128, 2, C], BF16, tag="P_s", bufs=1)
            PT8 = wk.tile([128, 2, C], BF16, tag="PT8", bufs=1)

            if DBG:
                nc.gpsimd.dma_start(dbg["qbT"][:], outbT["q"][:])
                nc.gpsimd.dma_start(dbg["kbT"][:], outbT["k"][:])
            # att logits + softmax over e (free axis)
            for g in range(2):
                ps = pf.tile([128, 512], F32, tag="pf")
                for hh in range(4):
                    h = g * 4 + hh
                    nc.tensor.matmul(
                        ps[:, TS(hh, 128)],
                        outbT["q"][:, TS(h, 128)],
                        outbT["k"][:, TS(h, 128)],
                        start=True,
                        stop=True,
                    )
                nc.scalar.activation(
                    out=ex_att[:, DS(g * 4, 4), :], in_=ps[:], func=ACT.Exp,
                    scale=float(SCALE),
                )
            with nc.allow_low_precision(reason="att softmax denom"):
                nc.vector.tensor_reduce(
                    asum[:], ex_att[:], axis=AX.X, op=ALU.add
                )
            nc.vector.reciprocal(arec[:], asum[:])
            for h in range(H):
                nc.vector.tensor_scalar_mul(
                    att_s[:, h, :], ex_att[:, h, :], arec[:, h : h + 1]
                )
            # attT
            for g in range(2):
                ps = trr.tile([128, 4, 128], BF16, tag="trr")
                for hh in range(4):
                    h = g * 4 + hh
                    nc.tensor.transpose(ps[:, hh, :], att_s[:, h, :], ident_b[:])
                rot_copy(attT[:, DS(g * 4, 4), :], ps[:], engines="vs")
            # B_h = att_h @ Wv_h  (lhsT = attT)
            for g in range(4):
                ps = pf.tile([128, 512], F32, tag="pf")
                for hh in range(2):
                    h = g * 2 + hh
                    nc.tensor.matmul(
                        ps[:, TS(hh, 256)],
                        attT[:, h, :],
                        wv[:, h, :],
                        start=True,
                        stop=True,
                    )
                rot_copy(
                    Bt[:, DS(g * 2, 2), :].rearrange("p a c -> p (a c)"), ps[:]
                )
            # P = sum_h Wp_h @ B_h   [2x128, 256]
            pps = pf.tile([128, 512], F32, tag="pf")
            for j in range(2):
                for h in range(H):
                    nc.tensor.matmul(
                        pps[:, TS(j, 256)],
                        wpT[:, h, TS(j, 128)],
                        Bt[:, h, :],
                        start=(h == 0),
                        stop=(h == 7),
                    )
            nc.vector.tensor_copy(
                P_s[:].rearrange("p a c -> p (a c)"), pps[:]
            )
            # P^T (4 block transposes), PT8 = SP * P^T
            ps = trr.tile([128, 4, 128], BF16, tag="trr")
            for j in range(2):
                for jj in range(2):
                    nc.tensor.transpose(
                        ps[:, j * 2 + jj, :], P_s[:, j, TS(jj, 128)], ident_b[:]
                    )
            dst = PT8[:].rearrange("p jj (j m) -> p j jj m", j=2)
            src = ps[:].rearrange("p (j jj) m -> p j jj m", j=2)
            nc.vector.tensor_copy(dst, src)

            if DBG:
                nc.gpsimd.dma_start(dbg["pt8"][:], PT8[:].rearrange("p a c -> p (a c)"))
            # final: out = relu(x @ P.T + bp); psum carries SP, undone by
            # the relu's scale. Two token tiles per psum bank.
            for tp in range(NT // 2):
                ps = pf.tile([128, 512], F32, tag="pf")
                for u in range(2):
                    t = tp * 2 + u
                    for j in range(2):
                        nc.tensor.matmul(
                            ps[:, DS(u * 256, 256)],
                            xTb[:, j, TS(t, 128)],
                            PT8[:, j, :],
                            start=(j == 0),
                            stop=False,
                        )
                    nc.tensor.matmul(
                        ps[:, DS(u * 256, 256)], ones_b[:], bp_sb[:],
                        start=False, stop=True,
                    )
                ob = wk.tile([128, 2, C], F32, tag="ob", bufs=4)
                nc.scalar.activation(
                    out=ob[:], in_=ps[:], func=ACT.Relu, scale=float(1.0 / SP)
                )
                nc.sync.dma_start(
                    out_d[DS(tp * 256, 256), :].rearrange("(a p) c -> p a c", p=128),
                    ob[:],
                )

    cap_waits(nc, nop_templates)
    return nc


_NC_CACHE = None


def _get_module():
    global _NC_CACHE
    if _NC_CACHE is None:
        _NC_CACHE = build_module()
    return _NC_CACHE


def _in_maps(inputs):
    x = np.ascontiguousarray(inputs["x"], dtype=np.float32)
    shared = {
        "Wq": np.ascontiguousarray(inputs["Wq"], dtype=np.float32),
        "Wk": np.ascontiguousarray(inputs["Wk"], dtype=np.float32),
        "Wv": np.ascontiguousarray(inputs["Wv"], dtype=np.float32),
        "Wp": np.ascontiguousarray(inputs["Wp"], dtype=np.float32),
        "bp": np.ascontiguousarray(inputs["bp"], dtype=np.float32).reshape(1, C),
    }
    return [{"x": x[b], **shared} for b in range(B)]


def kernel(**inputs) -> np.ndarray:
    nc = _get_module()
    res = run_bass_kernel_spmd(nc, _in_maps(inputs), core_ids=list(range(B)))
    return np.stack([res.results[b]["out"] for b in range(B)], axis=0)


def run_traced(**inputs):
    """kernel() with NTFF tracing; returns (output, BassKernelResults)."""
    nc = _get_module()
    res = run_bass_kernel_spmd(
        nc, _in_maps(inputs), core_ids=list(range(B)), trace=True
    )
    out = np.stack([res.results[b]["out"] for b in range(B)], axis=0)
    return out, res


# revision 27
# speedup vs baseline: 2.2726x; 1.1384x over previous
"""Trainium2 Bass kernel for nn_Attention_36481452212797.

Contract: kernel(**inputs) takes FULL inputs
  x [8, 4096, 256] f32, Wq/Wk/Wv [1024, 256], Wp [256, 1024], bp [256]
and returns the FULL output [8, 4096, 256] f32.

Sharding: data-parallel over B — one batch sample per NeuronCore, no
collectives.

Algorithm (factored): q/k are rank-C products (q = x @ Wq.T), so every DTA
stage matmul is factored through the C=256 dim:
    z      = softmax_KC(x @ G)         with G = Wq.T @ bases      [C, KC]
    basesT = l2norm(S @ Wq.T)          with S = z.T @ x           [KC, C]
which cuts stage FLOPs ~3.2x vs materializing [N, 4C] streams. Only the
seed (bases0 = l2norm(maxpool(W @ x.T))) needs the full [4C, N] product.
The whole v-projection + cluster attention + output projection collapses
into a single [C, C] matrix
    P = sum_h Wp_h @ att_h @ Wv_h,   out = relu(x @ P.T + bp)
replacing ~5.4 GFLOP with ~0.74 GFLOP per sample.

The per-stage l2norm of bases is deferred: bases stay unnormalized and the
1/(1e-6+norm) factor rides as a per-partition scalar on the next stage's
G^T (partition axis = KC). The reference's l2norm of z over N cancels into
the bases l2norm (numpy-validated, O(1e-8)).

Precision: the big x-sided matmuls (seed, z, S^T, ybT, final) run in
fp8-e4m3 with DoubleRow perf mode (2 K-tiles per instruction); smaller
ones in bf16. fp8 operands carry power-of-2 scales (W^T x256, z x16,
S x64, P^T x32768) that are unwound exactly in ACT scale parameters.
End-to-end maxabs/scale vs the fp32 reference: ~2.3e-3 (numpy-validated).

Engine budget: the seed maxpool can ONLY run on DVE (Pool has no max
ucode and cannot touch PSUM; ACT has no max), so DVE gets the seed
reduces and softmax sums; ACT drains most other PSUM traffic (activation
`scale=` does the normalization muls for free); Pool does the SBUF-side
softmax muls via stride-0 broadcast.
"""

import copy
import sys
from contextlib import ExitStack

import numpy as np

sys.path.insert(0, "/opt/trn_rl_repo")

import concourse.bass as bass
import concourse.mybir as mybir
import concourse.tile as tile
from concourse.bass_utils import run_bass_kernel_spmd
from concourse.masks import make_identity

B, N, C, H, KC, STAGES = 8, 4096, 256, 8, 128, 3
C4 = 4 * C          # 1024
HD = C4 // H        # 128
SCALE = (C // H) ** -0.5
NT = N // 128       # 32 token tiles
W = N // KC         # 32: maxpool window

F32 = mybir.dt.float32
BF16 = mybir.dt.bfloat16
FP8 = mybir.dt.float8e4
AX = mybir.AxisListType
ALU = mybir.AluOpType
ACT = mybir.ActivationFunctionType
DR = mybir.MatmulPerfMode.DoubleRow
DS = bass.ds
TS = bass.ts

SW = 256.0     # wT8 = SW * W^T (fp8; seeds + z-logit matmuls only)
SZ = 1.0       # z stays bf16
SS = 1.0       # S^T stays bf16
SP = 1.0       # P^T stays bf16


def cap_waits(nc, nop_templates, max_waits=1):
    """The walrus build here rejects instructions carrying more than one
    sync-wait command. Move excess waits onto EVSEM no-op carriers inserted
    before the capped instruction on the same engine."""
    m = nc.m
    new_m = copy.replace(m, functions=[])
    n_carriers = 0
    for function in m.functions:
        new_f = copy.replace(function, blocks=[])
        new_f.set_allocations_from_list(function.allocations)
        for block in function.blocks:
            new_insts = []
            for inst in block.instructions:
                si = inst.sync_info
                if si is not None and si.on_wait and len(si.on_wait) > max_waits:
                    waits = list(si.on_wait)
                    for w in waits[: len(waits) - max_waits]:
                        nop = copy.replace(
                            nop_templates[inst.engine],
                            name=f"{inst.name}-wc{n_carriers}",
                        )
                        tsi = nop_templates[inst.engine].sync_info
                        nop.sync_info = mybir.SyncInfo(
                            on_wait=[w],
                            on_update=list(tsi.on_update) if tsi else [],
                        )
                        new_insts.append(nop)
                        n_carriers += 1
                    inst.sync_info = mybir.SyncInfo(
                        on_wait=waits[len(waits) - max_waits :],
                        on_update=list(si.on_update or []),
                    )
                new_insts.append(inst)
            new_block = copy.replace(block, instructions=new_insts)
            new_f.blocks.append(new_block)
        new_m.functions.append(new_f)
    nc.m = new_m
    return n_carriers


def build_module():
    nc = bass.Bass()
    _dummy = nc.alloc_semaphore("waitcap_dummy")
    nop_templates = {
        e.ins.engine: e.ins
        for e in (
            nc.tensor.sem_inc(_dummy, 0),
            nc.vector.sem_inc(_dummy, 0),
            nc.scalar.sem_inc(_dummy, 0),
            nc.gpsimd.sem_inc(_dummy, 0),
            nc.sync.sem_inc(_dummy, 0),
        )
    }

    x_d = nc.declare_dram_parameter("x", [N, C], F32, isOutput=False)
    w_d = {
        "q": nc.declare_dram_parameter("Wq", [C4, C], F32, isOutput=False),
        "k": nc.declare_dram_parameter("Wk", [C4, C], F32, isOutput=False),
        "v": nc.declare_dram_parameter("Wv", [C4, C], F32, isOutput=False),
    }
    wp_d = nc.declare_dram_parameter("Wp", [C, C4], F32, isOutput=False)
    bp_d = nc.declare_dram_parameter("bp", [1, C], F32, isOutput=False)
    out_d = nc.declare_dram_parameter("out", [N, C], F32, isOutput=True)
    DBG = False
    if DBG:
        dbg = {
            nm: nc.declare_dram_parameter(f"dbg_{nm}", [128, sz], F32, isOutput=True)
            for nm, sz in [
                ("mxq", 8 * KC), ("bnrq", C4), ("g8q", 2 * KC), ("sumq", NT),
                ("ztq", NT * KC), ("st8q", 2 * KC), ("qbT", C4), ("kbT", C4),
                ("pt8", 2 * C),
            ]
        }

    with tile.TileContext(nc) as tc, ExitStack() as ctx:
        consts = ctx.enter_context(tc.tile_pool(name="consts", bufs=1))
        main = ctx.enter_context(tc.tile_pool(name="main", bufs=1))
        wk = ctx.enter_context(tc.tile_pool(name="wk", bufs=2))

        ident_f = consts.tile([128, 128], F32)
        make_identity(nc, ident_f[:])
        ident_b = consts.tile([128, 128], BF16)
        nc.vector.tensor_copy(ident_b[:], ident_f[:])
        eps_ap = {}
        for nm, v in (("sw", SW * 1e-6), ("yb", 1e-6)):
            t = consts.tile([128, 1], F32, name=f"eps_{nm}", tag=f"eps_{nm}")
            nc.vector.memset(t[:], float(v))
            eps_ap[nm] = t

        ones_b = consts.tile([1, 128], BF16)
        nc.vector.memset(ones_b[:], 1.0)
        bp_f = consts.tile([1, C], F32)
        nc.sync.dma_start(bp_f[:], bp_d[:])
        bp_sb = consts.tile([1, C], BF16)
        with nc.allow_low_precision(reason="bias prescale"):
            nc.vector.tensor_scalar_mul(bp_sb[:], bp_f[:], float(SP))

        # ---- persistent tensors ----
        x8 = main.tile([128, NT, C], FP8, tag="x8")
        xb = main.tile([128, NT, C], BF16, tag="xb")
        xT8 = main.tile([128, 2, N], FP8, tag="xT8")
        xTb = main.tile([128, 2, N], BF16, tag="xTb")
        wnat = {s: main.tile([128, 8, C], BF16, tag=f"wnat_{s}", name=f"wnat_{s}") for s in "qk"}
        wT8 = {s: main.tile([128, 2, C4], FP8, tag=f"wT8_{s}", name=f"wT8_{s}") for s in "qk"}
        wTb = {s: main.tile([128, 2, C4], BF16, tag=f"wTb_{s}", name=f"wTb_{s}") for s in "qk"}
        wv = main.tile([128, 8, C], BF16, tag="wv")
        wpT = main.tile([128, 8, C], BF16, tag="wpT")
        outbT = {"q": main.tile([128, C4], BF16, tag="qbT", name="qbT"),
                 "k": main.tile([128, C4], BF16, tag="kbT", name="kbT")}
        mx = {s: main.tile([128, 8, KC], BF16, tag=f"mx_{s}", name=f"mx_{s}") for s in "qk"}
        zt = {s: main.tile([128, NT, KC], BF16, tag=f"z_{s}", name=f"z_{s}") for s in "qk"}
        ext = {s: main.tile([128, NT, KC], BF16, tag=f"ex_{s}", name=f"ex_{s}") for s in "qk"}
        bnr = {s: main.tile([128, C4], BF16, tag=f"bnr_{s}", name=f"bnr_{s}") for s in "qk"}
        basesN = {s: main.tile([128, 8, KC], BF16, tag=f"bN_{s}", name=f"bN_{s}") for s in "qk"}
        sqscr = {s: main.tile([128, C4], BF16, tag=f"sq_{s}", name=f"sq_{s}") for s in "qk"}
        G8 = {s: main.tile([128, 2, KC], FP8, tag=f"G_{s}", name=f"G_{s}") for s in "qk"}
        ST8 = {s: main.tile([128, 2, KC], BF16, tag=f"ST_{s}", name=f"ST_{s}") for s in "qk"}
        sums = {s: main.tile([128, NT], BF16, tag=f"sm_{s}", name=f"sm_{s}") for s in "qk"}
        recb = {s: main.tile([128, NT], BF16, tag=f"rb_{s}", name=f"rb_{s}") for s in "qk"}
        ssq = {s: main.tile([128, 1], F32, tag=f"ssq_{s}", name=f"ssq_{s}") for s in "qk"}
        rnrm = {s: main.tile([128, 1], F32, tag=f"rn_{s}", name=f"rn_{s}") for s in "qk"}
        rnrmG = {s: main.tile([128, 1], F32, tag=f"rg_{s}", name=f"rg_{s}") for s in "qk"}

        _rot = [0]

        def rot_copy(dst_ap, src_ap, engines="vs", scale=None):
            """Rotate PSUM->SBUF drains across DVE(v)/ACT(s)."""
            e = engines[_rot[0] % len(engines)]
            _rot[0] += 1
            if scale is not None:
                if e == "v":
                    nc.vector.tensor_scalar_mul(dst_ap, src_ap, float(scale))
                else:
                    nc.scalar.activation(
                        out=dst_ap, in_=src_ap, func=ACT.Copy, scale=float(scale)
                    )
            elif e == "v":
                nc.vector.tensor_copy(dst_ap, src_ap)
            else:
                nc.scalar.copy(dst_ap, src_ap)

        # ================= phase 0: loads + transposes =================
        with ExitStack() as p0:
            tr8 = p0.enter_context(tc.tile_pool(name="tr8", bufs=2, space="PSUM"))
            trb = p0.enter_context(tc.tile_pool(name="trb", bufs=2, space="PSUM"))
            trf = p0.enter_context(tc.tile_pool(name="trf", bufs=2, space="PSUM"))

            # loads (gpsimd = SWDGE casting DMA; sync = plain f32)
            def load_x(c):
                nc.gpsimd.dma_start(
                    xb[:, DS(c * 8, 8), :],
                    x_d[DS(c * 1024, 1024), :].rearrange("(a p) c -> p a c", p=128),
                )
                nc.gpsimd.dma_start(
                    x8[:, DS(c * 8, 8), :],
                    x_d[DS(c * 1024, 1024), :].rearrange("(a p) c -> p a c", p=128),
                )

            def load_w(s, dst):
                nc.gpsimd.dma_start(
                    dst, w_d[s][:].rearrange("(a p) c -> p a c", p=128)
                )

            load_x(0)
            load_x(1)
            load_w("q", wnat["q"][:])
            load_x(2)
            load_x(3)
            load_w("k", wnat["k"][:])
            load_w("v", wv[:])
            wp_stage = wk.tile([128, 2, C4], F32, tag="wp_stage", bufs=1)
            nc.sync.dma_start(
                wp_stage[:], wp_d[:].rearrange("(a p) c -> p a c", p=128)
            )

            # x transposes: xT8[:, j, t*128:(t+1)*128] = x8[:, t, j*128:..]^T
            for c in range(4):
                for j in range(2):
                    ps = tr8.tile([128, 1024], BF16, tag="tr8")
                    for a in range(8):
                        t = c * 8 + a
                        nc.tensor.transpose(
                            ps[:, TS(a, 128)], xb[:, t, TS(j, 128)], ident_b[:]
                        )
                    rot_copy(xT8[:, j, DS(c * 1024, 1024)], ps[:])
                    rot_copy(xTb[:, j, DS(c * 1024, 1024)], ps[:])

            # weight transposes -> fp8 with xSW scale
            for s in "qk":
                for jj in range(2):
                    ps = trb.tile([128, 1024], BF16, tag="trb")
                    for i in range(8):
                        nc.tensor.transpose(
                            ps[:, TS(i, 128)], wnat[s][:, i, TS(jj, 128)], ident_b[:]
                        )
                    rot_copy(wT8[s][:, jj, :], ps[:], scale=SW)
                    rot_copy(wTb[s][:, jj, :], ps[:])

            # Wp transposes: wpT[:, j, a*128:..] = wp_stage[:, a, j*128:..]^T
            for a in range(2):
                for jg in range(2):
                    ps = trf.tile([128, 4, 128], F32, tag="trf")
                    for u in range(4):
                        j = jg * 4 + u
                        nc.tensor.transpose(
                            ps[:, u, :], wp_stage[:, a, TS(j, 128)], ident_f[:]
                        )
                    rot_copy(wpT[:, DS(jg * 4, 4), TS(a, 128)], ps[:])

        # ========= phases 1+2: seeds + DTA stages (one PSUM layout) =====
        # The seed maxpool is a 76us DVE-serial job; to keep DVE dense the
        # k-stream's seed groups are interleaved into q's stage pipeline
        # (and the stages of the two streams are interleaved with each
        # other). One shared PSUM layout (8 banks):
        #   sdyb [128,1024]f32 x2 (seed groups + ybT)  = 4 banks
        #   trb2 [128,1024]bf16 x1, tg8/zp/stp [128,512] x1 each = 4 banks
        with ExitStack() as p12:
            sdyb = p12.enter_context(tc.tile_pool(name="sdyb", bufs=2, space="PSUM"))
            trb2 = p12.enter_context(tc.tile_pool(name="trb2", bufs=1, space="PSUM"))
            zp = p12.enter_context(tc.tile_pool(name="zp", bufs=2, space="PSUM"))
            stp = p12.enter_context(tc.tile_pool(name="stp", bufs=1, space="PSUM"))

            def seed_group(s, m, qtr):
                """One [128,1024] seed psum group + its DVE maxpool reduce."""
                ps = sdyb.tile([128, 1024], F32, tag="sdyb", name="ps_sd")
                for u in range(2):
                    nc.tensor.matmul(
                        ps[:, TS(u, 512)],
                        wT8[s][:, :, TS(m, 128)],
                        xT8[:, :, DS(qtr * 1024 + u * 512, 512)],
                        start=True,
                        stop=True,
                        perf_mode=DR,
                    )
                nc.vector.tensor_reduce(
                    mx[s][:, m, DS(qtr * 32, 32)],
                    ps[:].rearrange("p (k w) -> p k w", w=W),
                    axis=AX.X,
                    op=ALU.max,
                )

            def norm_chain(s, src_ap, sc, eps):
                """rnrmG[s] = (1/sc)/(1e-6 + sqrt(sum(src^2))/sc) — the drain
                scale yielding l2-normalized bases from an sc-scaled source
                (per-partition AP; partition = KC). 3 ACT ops, no engine
                hops: 1/(u + sc*eps) == (1/sc)/(eps + u/sc) for u=sc*nrm."""
                nc.scalar.activation(
                    out=sqscr[s][:], in_=src_ap, func=ACT.Square,
                    accum_out=ssq[s][:],
                )
                nc.scalar.activation(
                    out=rnrm[s][:], in_=ssq[s][:], func=ACT.Sqrt, bias=eps[:]
                )
                nc.vector.reciprocal(rnrmG[s][:], rnrm[s][:])

            def bases0(s):
                """mx (SW-scaled) -> bnr = l2-normalized basesT. The norm is
                computed from the raw transposed copy, then applied in
                place (per-partition scalar; partition = KC)."""
                ps = trb2.tile([128, 1024], BF16, tag="trb2")
                for i in range(8):
                    nc.tensor.transpose(
                        ps[:, TS(i, 128)], mx[s][:, i, :], ident_b[:]
                    )
                nc.scalar.copy(bnr[s][:], ps[:])
                norm_chain(s, bnr[s][:], SW, eps_ap["sw"])
                nc.vector.tensor_scalar_mul(bnr[s][:], bnr[s][:], rnrmG[s][:])

            def pool_sum_tree(s, p):
                """Softmax denominators for 8 token tiles on Pool (7-level
                binary adds) to offload DVE."""
                t1 = wk.tile([128, 8, 64], BF16, tag="t1", bufs=2, name="t1")
                t2 = wk.tile([128, 8, 32], BF16, tag="t2", bufs=2, name="t2")
                t3 = wk.tile([128, 8, 16], BF16, tag="t3", bufs=2, name="t3")
                t4 = wk.tile([128, 8, 8], BF16, tag="t4", bufs=2, name="t4")
                t5 = wk.tile([128, 8, 4], BF16, tag="t5", bufs=2, name="t5")
                t6 = wk.tile([128, 8, 2], BF16, tag="t6", bufs=2, name="t6")
                ex8 = ext[s][:, DS(p * 8, 8), :]
                ad = nc.gpsimd.tensor_add
                with nc.allow_low_precision(reason="softmax denom"):
                    ad(t1[:], ex8[:, :, 0:64], ex8[:, :, 64:128])
                    ad(t2[:], t1[:, :, 0:32], t1[:, :, 32:64])
                    ad(t3[:], t2[:, :, 0:16], t2[:, :, 16:32])
                    ad(t4[:], t3[:, :, 0:8], t3[:, :, 8:16])
                    ad(t5[:], t4[:, :, 0:4], t4[:, :, 4:8])
                    ad(t6[:], t5[:, :, 0:2], t5[:, :, 2:4])
                    ad(sums[s][:, DS(p * 8, 8)], t6[:, :, 0], t6[:, :, 1])

            def stage(s, i, ve, mode="spread"):
                # hide: everything off-DVE where possible except sums
                # spread: sums alternate DVE/Pool-tree, muls all on Pool
                hide_ = mode == "hide"
                """Generator: yields at pipeline checkpoints so the caller
                can interleave other work into the emission order. `ve` is
                the drain engine (nc.vector or nc.scalar) for this stage.

                bnr holds l2-NORMALIZED basesT (the norm was folded into
                its drain), so G8 = SW/4 * sum_ch W_ch.T @ bases_ch comes
                straight out of a matmul with a plain scaled drain.
                """
                last = i == STAGES - 1
                hide = hide_

                def drain(dst, src, scale=None):
                    if scale is None:
                        if ve is nc.vector:
                            nc.vector.tensor_copy(dst, src)
                        else:
                            nc.scalar.copy(dst, src)
                    elif ve is nc.vector:
                        nc.vector.tensor_scalar_mul(dst, src, scale)
                    else:
                        nc.scalar.activation(
                            out=dst, in_=src, func=ACT.Copy, scale=scale
                        )

                # -- basesN [c4, KC] = transpose(bnr) --
                ps = trb2.tile([128, 1024], BF16, tag="trb2")
                for ch in range(8):
                    nc.tensor.transpose(
                        ps[:, TS(ch, 128)], bnr[s][:, TS(ch, 128)], ident_b[:]
                    )
                drain(basesN[s][:].rearrange("p a k -> p (a k)"), ps[:])
                # -- G8 = SG * sum_ch W_ch.T @ basesN_ch  [C, KC] --
                gps = zp.tile([128, 512], F32, tag="zp")
                for j in range(2):
                    for ch in range(8):
                        nc.tensor.matmul(
                            gps[:, DS(j * 128, 128)],
                            wnat[s][:, ch, TS(j, 128)],
                            basesN[s][:, ch, :],
                            start=(ch == 0),
                            stop=(ch == 7),
                        )
                drain(
                    G8[s][:].rearrange("p a k -> p (a k)"), gps[:, 0:256],
                    scale=float(SW / 4.0),
                )
                yield
                # -- z8 = SZ * softmax_KC(x @ G); z psum carries SW --
                for g in range(8):
                    ps = zp.tile([128, 512], F32, tag="zp")
                    for tt in range(4):
                        t = g * 4 + tt
                        nc.tensor.matmul(
                            ps[:, TS(tt, 128)],
                            xT8[:, :, TS(t, 128)],
                            G8[s][:],
                            start=True,
                            stop=True,
                            perf_mode=DR,
                        )
                    nc.scalar.activation(
                        out=ext[s][:, DS(g * 4, 4), :], in_=ps[:], func=ACT.Exp,
                        scale=float(4.0 / SW),
                    )
                    if g % 2 == 1:
                        p = g // 2
                        if False:
                            pool_sum_tree(s, p)
                        else:
                            with nc.allow_low_precision(reason="softmax denom"):
                                nc.vector.tensor_reduce(
                                    sums[s][:, DS(p * 8, 8)],
                                    ext[s][:, DS(p * 8, 8), :],
                                    axis=AX.X,
                                    op=ALU.add,
                                )
                        with nc.allow_low_precision(reason="softmax recip"):
                            nc.vector.reciprocal(
                                recb[s][:, DS(p * 8, 8)], sums[s][:, DS(p * 8, 8)]
                            )
                            nc.vector.tensor_scalar_mul(
                                recb[s][:, DS(p * 8, 8)],
                                recb[s][:, DS(p * 8, 8)],
                                float(SZ),
                            )
                        for hh in range(2):
                            o = p * 8 + hh * 4
                            rb = (
                                recb[s][:, DS(o, 4)]
                                .rearrange("p a -> p a ()")
                                .broadcast_to((128, 4, KC))
                            )
                            meng = nc.gpsimd if hide else nc.vector
                            meng.tensor_mul(
                                zt[s][:, DS(o, 4), :],
                                ext[s][:, DS(o, 4), :],
                                rb,
                            )
                        yield
                # -- ST8 = SS * (sum_t x_t.T @ z_t); S psum carries SZ --
                sps = stp.tile([128, 512], F32, tag="stp")
                for t in range(NT):
                    for j in range(2):
                        nc.tensor.matmul(
                            sps[:, TS(j, 128)],
                            xb[:, t, TS(j, 128)],
                            zt[s][:, t, :],
                            start=(t == 0),
                            stop=(t == NT - 1),
                        )
                nc.scalar.copy(
                    ST8[s][:].rearrange("p a k -> p (a k)"), sps[:, 0:256]
                )
                yield

                # -- ybT = S @ W.T [KC, C4]; psum carries SS*SW --
                yps = sdyb.tile([128, 1024], F32, tag="sdyb", name="ps_yb")
                for hh in range(2):
                    for j in range(2):
                        nc.tensor.matmul(
                            yps[:, TS(hh, 512)],
                            ST8[s][:, j, :],
                            wTb[s][:, j, DS(hh * 512, 512)],
                            start=(j == 0),
                            stop=(j == 1),
                        )
                ysc = 1.0
                norm_chain(s, yps[:], ysc, eps_ap["yb"])
                dst = outbT[s] if last else bnr[s]
                if ve is nc.vector:
                    nc.vector.tensor_scalar_mul(dst[:], yps[:], rnrmG[s][:])
                else:
                    nc.scalar.activation(
                        out=dst[:], in_=yps[:], func=ACT.Copy, scale=rnrmG[s][:]
                    )
                yield

            def pump(gen, n=1):
                """Advance a stage generator n checkpoints; True if alive."""
                if gen is None:
                    return None
                for _ in range(n):
                    if next(gen, "done") == "done":
                        return None
                return gen

            # q seeds first, then q's stages overlap k's seeds; the two
            # streams' stages interleave checkpoint-by-checkpoint after.
            kseeds = [(m, qtr) for qtr in range(4) for m in range(8)]
            for qtr in range(4):
                for m in range(8):
                    seed_group("q", m, qtr)
            if DBG:
                nc.gpsimd.dma_start(dbg["mxq"][:], mx["q"][:].rearrange("p a k -> p (a k)"))
            bases0("q")
            if DBG:
                nc.gpsimd.dma_start(dbg["bnrq"][:], bnr["q"][:])
            gq = stage("q", 0, nc.scalar, "hide")
            for gi, (m, qtr) in enumerate(kseeds):
                seed_group("k", m, qtr)
                if gi % 2 == 1 and gq is not None:
                    gq = pump(gq)
            while gq is not None:
                gq = pump(gq)
            if DBG:
                nc.gpsimd.dma_start(dbg["g8q"][:], G8["q"][:].rearrange("p a k -> p (a k)"))
                nc.gpsimd.dma_start(dbg["sumq"][:], sums["q"][:])
                nc.gpsimd.dma_start(dbg["ztq"][:], zt["q"][:].rearrange("p a k -> p (a k)"))
                nc.gpsimd.dma_start(dbg["st8q"][:], ST8["q"][:].rearrange("p a k -> p (a k)"))
            bases0("k")
            pairs = [("k", 0, "q", 1), ("k", 1, "q", 2)]
            for sa, ia, sb, ib in pairs:
                ga = stage(sa, ia, nc.scalar, "spread")
                gb = stage(sb, ib, nc.scalar, "spread")
                ga = pump(ga, 3)
                while ga is not None or gb is not None:
                    ga = pump(ga)
                    gb = pump(gb)
            gk = stage("k", 2, nc.scalar, "spread")
            while gk is not None:
                gk = pump(gk)

        # ================= phase 3: attention + P + final ==============
        with ExitStack() as p3:
            pf = p3.enter_context(tc.tile_pool(name="pf", bufs=3, space="PSUM"))
            trr = p3.enter_context(tc.tile_pool(name="trr", bufs=2, space="PSUM"))

            ex_att = wk.tile([128, 8, KC], BF16, tag="ex_att", bufs=1)
            asum = wk.tile([128, 8], BF16, tag="asum", bufs=1)
            arec = wk.tile([128, 8], F32, tag="arec", bufs=1)
            att_s = wk.tile([128, 8, KC], BF16, tag="att_s", bufs=1)
            attT = wk.tile([128, 8, KC], BF16, tag="attT", bufs=1)
            Bt = wk.tile([128, 8, C], BF16, tag="Bt", bufs=1)
            P_s = wk.tile([128, 2, C], BF16, tag="P_s", bufs=1)
            PT8 = wk.tile([128, 2, C], BF16, tag="PT8", bufs=1)

            if DBG:
                nc.gpsimd.dma_start(dbg["qbT"][:], outbT["q"][:])
                nc.gpsimd.dma_start(dbg["kbT"][:], outbT["k"][:])
            # att logits + softmax over e (free axis)
            for g in range(2):
                ps = pf.tile([128, 512], F32, tag="pf")
                for hh in range(4):
                    h = g * 4 + hh
                    nc.tensor.matmul(
                        ps[:, TS(hh, 128)],
                        outbT["q"][:, TS(h, 128)],
                        outbT["k"][:, TS(h, 128)],
                        start=True,
                        stop=True,
                    )
                nc.scalar.activation(
                    out=ex_att[:, DS(g * 4, 4), :], in_=ps[:], func=ACT.Exp,
                    scale=float(SCALE),
                )
            with nc.allow_low_precision(reason="att softmax denom"):
                nc.vector.tensor_reduce(
                    asum[:], ex_att[:], axis=AX.X, op=ALU.add
                )
            nc.vector.reciprocal(arec[:], asum[:])
            for h in range(H):
                nc.vector.tensor_scalar_mul(
                    att_s[:, h, :], ex_att[:, h, :], arec[:, h : h + 1]
                )
            # attT
            for g in range(2):
                ps = trr.tile([128, 4, 128], BF16, tag="trr")
                for hh in range(4):
                    h = g * 4 + hh
                    nc.tensor.transpose(ps[:, hh, :], att_s[:, h, :], ident_b[:])
                rot_copy(attT[:, DS(g * 4, 4), :], ps[:], engines="vs")
            # B_h = att_h @ Wv_h  (lhsT = attT)
            for g in range(4):
                ps = pf.tile([128, 512], F32, tag="pf")
                for hh in range(2):
                    h = g * 2 + hh
                    nc.tensor.matmul(
                        ps[:, TS(hh, 256)],
                        attT[:, h, :],
                        wv[:, h, :],
                        start=True,
                        stop=True,
                    )
                rot_copy(
                    Bt[:, DS(g * 2, 2), :].rearrange("p a c -> p (a c)"), ps[:]
                )
            # P = sum_h Wp_h @ B_h   [2x128, 256]
            pps = pf.tile([128, 512], F32, tag="pf")
            for j in range(2):
                for h in range(H):
                    nc.tensor.matmul(
                        pps[:, TS(j, 256)],
                        wpT[:, h, TS(j, 128)],
                        Bt[:, h, :],
                        start=(h == 0),
                        stop=(h == 7),
                    )
            nc.vector.tensor_copy(
                P_s[:].rearrange("p a c -> p (a c)"), pps[:]
            )
            # P^T (4 block transposes), PT8 = SP * P^T
            ps = trr.tile([128, 4, 128], BF16, tag="trr")
            for j in range(2):
                for jj in range(2):
                    nc.tensor.transpose(
                        ps[:, j * 2 + jj, :], P_s[:, j, TS(jj, 128)], ident_b[:]
                    )
            dst = PT8[:].rearrange("p jj (j m) -> p j jj m", j=2)
            src = ps[:].rearrange("p (j jj) m -> p j jj m", j=2)
            nc.vector.tensor_copy(dst, src)

            if DBG:
                nc.gpsimd.dma_start(dbg["pt8"][:], PT8[:].rearrange("p a c -> p (a c)"))
            # final: out = relu(x @ P.T + bp); psum carries SP, undone by
            # the relu's scale. Two token tiles per psum bank.
            for tp in range(NT // 2):
                ps = pf.tile([128, 512], F32, tag="pf")
                for u in range(2):
                    t = tp * 2 + u
                    for j in range(2):
                        nc.tensor.matmul(
                            ps[:, DS(u * 256, 256)],
                            xTb[:, j, TS(t, 128)],
                            PT8[:, j, :],
                            start=(j == 0),
                            stop=False,
                        )
                    nc.tensor.matmul(
                        ps[:, DS(u * 256, 256)], ones_b[:], bp_sb[:],
                        start=False, stop=True,
                    )
                ob = wk.tile([128, 2, C], F32, tag="ob", bufs=4)
                nc.scalar.activation(
                    out=ob[:], in_=ps[:], func=ACT.Relu, scale=float(1.0 / SP)
                )
                nc.sync.dma_start(
                    out_d[DS(tp * 256, 256), :].rearrange("(a p) c -> p a c", p=128),
                    ob[:],
                )

    cap_waits(nc, nop_templates)
    return nc


_NC_CACHE = None


def _get_module():
    global _NC_CACHE
    if _NC_CACHE is None:
        _NC_CACHE = build_module()
    return _NC_CACHE


def _in_maps(inputs):
    x = np.ascontiguousarray(inputs["x"], dtype=np.float32)
    shared = {
        "Wq": np.ascontiguousarray(inputs["Wq"], dtype=np.float32),
        "Wk": np.ascontiguousarray(inputs["Wk"], dtype=np.float32),
        "Wv": np.ascontiguousarray(inputs["Wv"], dtype=np.float32),
        "Wp": np.ascontiguousarray(inputs["Wp"], dtype=np.float32),
        "bp": np.ascontiguousarray(inputs["bp"], dtype=np.float32).reshape(1, C),
    }
    return [{"x": x[b], **shared} for b in range(B)]


def kernel(**inputs) -> np.ndarray:
    nc = _get_module()
    res = run_bass_kernel_spmd(nc, _in_maps(inputs), core_ids=list(range(B)))
    return np.stack([res.results[b]["out"] for b in range(B)], axis=0)


def run_traced(**inputs):
    """kernel() with NTFF tracing; returns (output, BassKernelResults)."""
    nc = _get_module()
    res = run_bass_kernel_spmd(
        nc, _in_maps(inputs), core_ids=list(range(B)), trace=True
    )
    out = np.stack([res.results[b]["out"] for b in range(B)], axis=0)
    return out, res
